# revision 1
# baseline (speedup 1.0000x reference)
"""Trainium2 Bass kernel for DifferentiableToposAttention.

Math:
  Q = sigmoid(x @ Wq.T + bq); K = sigmoid(x @ Wk.T + bk); V = x @ Wv.T + bv
  truth[q,k] = mean_d min(1 - Q[q,d] + K[k,d], 1) = 1 - (1/D) sum_d relu(Q-K)
  logit[q,k] = 10*truth; masked (k>q) logits are 0 exactly (weight exp(0)=1).
  out[q,:] = sum_k softmax(logit)[q,k] * V[k,:]

Score via PWL-interpolated relu as a matmul (contraction D*T, T=8):
  relu(a-b) ~= sum_{m=1..T} G_m(a) * r_m(b)
  r_m(b) = relu(m/T - b)                     (ACT, one op per m)
  G_m(a) = L_{m-1}(a) - L_m(a),  G_T = L_{T-1},
  L_m(a) = clamp(T*a - m, 0, 1)              (DVE, two ops per m)
This is exact PWL interpolation of relu(a-b) on the a-grid {m/T}; the
only error is Q-side quantization (<= 1/(2T), active only for same-cell
pairs).  End-to-end rel err vs fp32 reference ~4e-3 at T=8.

Sharding: 8 cores; core c = (b=c//4, l=c%4) handles batch b, query
blocks gA=l (keys window [0,512)) and gB=7-l (window [0,1024)) -- the
same compile-time shapes on every core (SPMD), host routes data.
Scores in [k,q] orientation (PSUM [128k, q]) so the exp output E^T is
directly the AV stationary -- no transposes anywhere.  exp fuses the
logit scale/bias (ACT: exp(-10/D * s + 10)).  Causal mask applied
post-exp: E' = E*M1 + (1-M1), M1 host-built per core.  Keys beyond the
512 window of group A contribute an analytic suffix: one matmul adds
ones[q] (x) [suffix-V | count] into the AV PSUM, whose appended ones
column accumulates the softmax denominator alongside AV.
"""

import sys

for _p in ("/opt/trn_rl_repo",):
    if _p not in sys.path:
        sys.path.insert(0, _p)

import numpy as np

import concourse.bass as bass
import concourse.mybir as mybir
import concourse.tile as tile
from concourse import bacc
from concourse.bass import ts
from concourse.bass_utils import run_bass_kernel_spmd

F32 = mybir.dt.float32
FP16 = mybir.dt.float16
BF16 = mybir.dt.bfloat16
AF = mybir.ActivationFunctionType
ALU = mybir.AluOpType

B, S, D = 2, 1024, 128
NCORES = 8
T = 6          # PWL knots
NKB = 8        # key blocks of 128


def _kb_map(kb: int, masked: bool):
    """kb -> (bank, col offset in bank, q-width). Masked: kb<4 carries
    both query groups (w=256), kb>=4 only group B (w=128)."""
    if masked:
        if kb < 4:
            return kb // 2, (kb % 2) * 256, 256
        return 2, (kb - 4) * 128, 128
    return kb // 2, (kb % 2) * 256, 256


def _build_program(masked: bool) -> bass.Bass:
    nbanks = 3 if masked else 4
    nc = bacc.Bacc()

    # xin = [wqkv | xqT | xT]; two DMA chunks: chunk1 ends after xT_lo
    NXIN = 3 * D + 256 + S
    CH1 = 3 * D + 256 + 512
    xin_d = nc.declare_dram_parameter("xin", [D, NXIN], FP16, isOutput=False)
    # consts: col 0 = bq, 1 = bk, 2..T+1 = m/T biases, T+2 = 10.0
    cs_d = nc.declare_dram_parameter("cs", [D, T + 3], F32, isOutput=False)
    # aux = [bvb | m1], one later DMA (bf16)
    naux = 4 * D + (nbanks * 512 if masked else 0)
    aux_d = nc.declare_dram_parameter("aux", [128, naux], BF16, isOutput=False)
    out_d = nc.declare_dram_parameter("out", [D, 256], F32, isOutput=True)

    with tile.TileContext(nc) as tc:
        with tc.tile_pool(name="singles", bufs=1) as singles:
            ones_col = singles.tile([128, 1], BF16)
            nc.vector.memset(ones_col[:], 1.0)
            ones_row = singles.tile([1, 128], BF16)
            nc.vector.memset(ones_row[:], 1.0)
            cs_sb = singles.tile([128, T + 3], F32)
            warm = singles.tile([128, 2], F32)

            xin = singles.tile([128, NXIN], FP16)
            aux = singles.tile([128, naux], BF16)
            KTb = singles.tile([128, S], FP16)
            QTb = singles.tile([128, 256], FP16)
            r_all = singles.tile([128, T, S], BF16)      # r_m(K)
            L_all = singles.tile([128, T, 256], BF16)    # clamp ramps of Q
            G_all = singles.tile([128, T, 256], BF16)    # tents of Q
            Vhat = singles.tile([128, NKB, D + 1], BF16)  # [V | 1]
            E_raw = singles.tile([128, nbanks, 512], BF16)
            out_s = singles.tile([128, 256], F32)
            rcpA = singles.tile([128, 1], F32)
            rcpB = singles.tile([128, 1], F32)
            if masked:
                E2 = singles.tile([128, nbanks, 512], BF16)
                m2_sb = singles.tile([128, nbanks * 512], BF16)
                sfx_row = singles.tile([1, D + 1], BF16)

            # preload the Sigmoid and Relu tables during the DMA wait (ACT
            # has two table slots; a load fires per function/scale switch)
            nc.vector.memset(warm[:], 0.0)
            nc.scalar.activation(warm[:, 0:1], warm[:, 0:1], AF.Sigmoid)
            nc.scalar.activation(
                warm[:, 0:1], warm[:, 0:1], AF.Relu, bias=0.0, scale=-1.0)
            nc.sync.dma_start(out=xin[:, 0:CH1], in_=xin_d[:, 0:CH1])
            nc.sync.dma_start(out=xin[:, CH1:NXIN], in_=xin_d[:, CH1:NXIN])
            nc.scalar.dma_start(out=cs_sb[:], in_=cs_d[:, :])
            nc.gpsimd.dma_start(out=aux[:], in_=aux_d[:, :])
            nc.vector.memset(Vhat[:, :, D:D + 1], 1.0)
            wq_sb = xin[:, 0:128]
            wk_sb = xin[:, 128:256]
            wv_sb = xin[:, 256:384]
            xqT = xin[:, 384:640]
            xT = xin[:, 640:640 + S]
            bvb4_sb = aux[:, 0:512]
            if masked:
                m1_sb = aux[:, 512:512 + nbanks * 512]
            bq_sb = cs_sb[:, 0:1]
            bk_sb = cs_sb[:, 1:2]

            with (
                tc.tile_pool(name="pp", bufs=3 if masked else 2,
                             space="PSUM") as pp,
                tc.tile_pool(name="pv", bufs=1, space="PSUM") as pv,
                tc.tile_pool(name="psc", bufs=1, space="PSUM") as pscp,
                tc.tile_pool(name="pav", bufs=1, space="PSUM") as pav,
            ):
                # ---- projections (PE order follows DMA chunk arrival) ----
                psK = pp.tile([128, 512], F32, tag="p")
                nc.tensor.matmul(psK[:], wk_sb, xT[:, 0:512])
                nc.scalar.activation(
                    KTb[:, 0:512], psK[:], AF.Sigmoid, bias=bk_sb, scale=1.0)
                psQ = pp.tile([128, 512], F32, tag="p")
                nc.tensor.matmul(psQ[:, 0:256], wq_sb, xqT)
                nc.scalar.activation(
                    QTb[:], psQ[:, 0:256], AF.Sigmoid, bias=bq_sb, scale=1.0)

                def v_blocks(half):
                    psV = pv.tile([128, 4, 128], F32, tag="v")
                    for i in range(4):
                        kb = half * 4 + i
                        nc.tensor.matmul(
                            psV[:, i, :], xT[:, ts(kb, 128)], wv_sb)
                    return psV

                # ---- encodings (lo half first: only needs sigK/sigQ) ----
                def emit_r(m, half):
                    dst = r_all[:, m - 1, 512 * half:512 * half + 512]
                    srcK = KTb[:, 512 * half:512 * half + 512]
                    if m in (1, 3):
                        nc.scalar.activation(
                            dst, srcK, AF.Relu,
                            bias=cs_sb[:, m + 1:m + 2], scale=-1.0)
                    else:
                        nc.vector.tensor_scalar(
                            dst, srcK, -1.0, float(m) / T, ALU.mult, ALU.add)
                        nc.vector.tensor_scalar(dst, dst, 0.0, None, ALU.max)

                def emit_L(m):
                    nc.vector.tensor_scalar(
                        L_all[:, m, :], QTb[:], float(T), float(-m),
                        ALU.mult, ALU.add)
                    nc.vector.tensor_scalar(
                        L_all[:, m, :], L_all[:, m, :], 0.0, 1.0,
                        ALU.max, ALU.min)

                emit_r(1, 0)
                emit_L(0)
                emit_L(1)
                nc.vector.tensor_sub(
                    G_all[:, 0, :], L_all[:, 0, :], L_all[:, 1, :])
                emit_r(2, 0)
                emit_r(3, 0)
                for m in range(2, T):
                    emit_L(m)
                    nc.vector.tensor_sub(
                        G_all[:, m - 1, :], L_all[:, m - 1, :], L_all[:, m, :])
                    if m + 2 <= T and (m + 2) not in (1, 3):
                        emit_r(m + 2, 0)
                nc.vector.tensor_copy(G_all[:, T - 1, :], L_all[:, T - 1, :])

                # hi-half projections + V blocks
                psV0 = v_blocks(0)
                psK2 = pp.tile([128, 512], F32, tag="p")
                nc.tensor.matmul(psK2[:], wk_sb, xT[:, 512:1024])
                nc.scalar.activation(
                    KTb[:, 512:1024], psK2[:], AF.Sigmoid, bias=bk_sb,
                    scale=1.0)
                psV1 = v_blocks(1)

                emit_r(1, 1)
                emit_r(3, 1)
                for m in (2, 4, 5, 6):
                    emit_r(m, 1)
                # Exp table preload with the tail scale/bias; input pinned
                # to the last ACT relu tile so it schedules after it
                nc.scalar.activation(
                    warm[:, 1:2], r_all[:, 2, 512:513], AF.Exp,
                    bias=cs_sb[:, T + 2:T + 3], scale=-10.0 / D)

                avall = pav.tile([128, 512], F32, tag="av")
                # V bias adds (DVE, after the r-lo/L/G critical chain)
                nc.vector.tensor_add(
                    Vhat[:, 0:4, 0:D], psV0[:], bvb4_sb[:])
                nc.vector.tensor_add(
                    Vhat[:, 4:8, 0:D], psV1[:], bvb4_sb[:])
                if masked:
                    # M2 = 1 - M1 on gpsimd (idle engine, SBUF only)
                    nc.gpsimd.tensor_scalar(
                        m2_sb[:], m1_sb[:], -1.0, 1.0, ALU.mult, ALU.add)

                # ---- score matmuls: lo banks first, then hi ----
                psc = []
                for bk_ in range(nbanks):
                    sc_bank = pscp.tile([128, 512], F32, tag=f"sc{bk_}")
                    psc.append(sc_bank)

                def score_mm(m, kb):
                    bank, off, w = _kb_map(kb, masked)
                    nc.tensor.matmul(
                        psc[bank][:, off:off + w],
                        r_all[:, m - 1, ts(kb, 128)],
                        G_all[:, m - 1, 256 - w:256],
                        start=(m == 1), stop=(m == T),
                        skip_group_check=True)

                def finish_bank(bank):
                    nc.scalar.activation(
                        E_raw[:, bank, :], psc[bank][:], AF.Exp,
                        bias=cs_sb[:, T + 2:T + 3], scale=-10.0 / D)
                    if masked:
                        nc.vector.tensor_mul(
                            E2[:, bank, :], E_raw[:, bank, :],
                            m1_sb[:, ts(bank, 512)])
                        nc.vector.tensor_add(
                            E2[:, bank, :], E2[:, bank, :],
                            m2_sb[:, ts(bank, 512)])

                for m in range(1, T + 1):
                    for kb in range(4):
                        score_mm(m, kb)
                finish_bank(0)
                finish_bank(1)
                if masked:
                    # suffix [sum V | 512] over key blocks 4..7 (shares the
                    # AV bank, disjoint columns)
                    pssfx = avall[0:1, 320:320 + D + 1]
                    for kb in range(4, 8):
                        nc.tensor.matmul(
                            pssfx, ones_col[:], Vhat[:, kb, :],
                            start=(kb == 4), stop=(kb == 7),
                            skip_group_check=True)
                    nc.scalar.copy(sfx_row[:], pssfx)
                for m in range(1, T + 1):
                    for kb in range(4, 8):
                        score_mm(m, kb)
                finish_bank(2)
                if not masked:
                    finish_bank(3)
                E_use = E2 if masked else E_raw

                # ---- suffix + AV + normalize ----
                avA = avall[:, 0:D + 1]
                avB = avall[:, D + 1:2 * D + 2]
                nblk_a = 4 if masked else 8
                for i, kb in enumerate(range(nblk_a)):
                    bank, off, w = _kb_map(kb, masked)
                    st = E_use[:, bank, off:off + 128]
                    nc.tensor.matmul(
                        avA, st, Vhat[:, kb, :],
                        start=(i == 0), stop=(not masked and kb == nblk_a - 1),
                        skip_group_check=True)
                if masked:
                    nc.tensor.matmul(
                        avA, ones_row[:], sfx_row[:],
                        start=False, stop=True, skip_group_check=True)
                for kb in range(NKB):
                    bank, off, w = _kb_map(kb, masked)
                    st = E_use[:, bank, off + w - 128:off + w]
                    nc.tensor.matmul(
                        avB, st, Vhat[:, kb, :],
                        start=(kb == 0), stop=(kb == NKB - 1),
                        skip_group_check=True)

                nc.vector.reciprocal(rcpA[:], avall[:, D:D + 1])
                nc.vector.tensor_scalar(
                    out_s[:, 0:128], avall[:, 0:D], rcpA[:], None, ALU.mult)
                nc.vector.reciprocal(rcpB[:], avall[:, 2 * D + 1:2 * D + 2])
                nc.vector.tensor_scalar(
                    out_s[:, 128:256], avall[:, D + 1:2 * D + 1], rcpB[:], None, ALU.mult)
                nc.sync.dma_start(out=out_d[:, :], in_=out_s[:])

    nc.finalize()
    return nc


_PROG_CACHE: dict[bool, bass.Bass] = {}


def _get_program(masked: bool) -> bass.Bass:
    if masked not in _PROG_CACHE:
        _PROG_CACHE[masked] = _build_program(masked)
    return _PROG_CACHE[masked]


def _build_m1(l: int) -> np.ndarray:
    """Post-exp causal mask, [k,q] orientation, bank-packed [128, 1536].
    1 = keep computed weight, 0 = masked (weight forced to exp(0)=1)."""
    gA, gB = l, 7 - l
    m1 = np.zeros((128, 3 * 512), dtype=np.float16)
    tri = (np.arange(128)[:, None] <= np.arange(128)[None, :])  # k<=q in blk
    for kb in range(8):
        bank, off, w = _kb_map(kb, True)
        base = bank * 512 + off
        units = [(gA, base), (gB, base + 128)] if w == 256 else [(gB, base)]
        for g, col in units:
            if kb < g:
                m1[:, col:col + 128] = 1.0
            elif kb == g:
                m1[:, col:col + 128] = tri
    return m1


def build_in_maps(x, Wq, bq, Wk, bk, Wv, bv, masked):
    wqkv = np.concatenate(
        [Wq.T.astype(np.float16), Wk.T.astype(np.float16),
         Wv.T.astype(np.float16)], axis=1)
    bvb = np.tile(bv.reshape(1, D).astype(np.float32), (D, 4))
    cs = np.zeros((D, T + 3), dtype=np.float32)
    cs[:, 0] = bq.astype(np.float32)
    cs[:, 1] = bk.astype(np.float32)
    cs[:, 2:T + 2] = np.arange(1, T + 1, dtype=np.float32) / T
    cs[:, T + 2] = 10.0
    cs = np.ascontiguousarray(cs)
    in_maps = []
    xTs = [x[b].T.astype(np.float16) for b in range(B)]
    for c in range(NCORES):
        b, l = divmod(c, 4)
        gA, gB = l, 7 - l
        xT = xTs[b]
        xqT = np.concatenate(
            [xT[:, 128 * gA:128 * gA + 128],
             xT[:, 128 * gB:128 * gB + 128]], axis=1)
        xin = np.ascontiguousarray(
            np.concatenate([wqkv, xqT, xT], axis=1))
        import ml_dtypes
        if masked:
            aux = np.concatenate(
                [bvb, _build_m1(l).astype(np.float32)], axis=1)
        else:
            aux = bvb
        aux16 = np.ascontiguousarray(aux.astype(ml_dtypes.bfloat16))
        im = {"xin": xin, "cs": cs, "aux": aux16}
        in_maps.append(im)
    return in_maps


def assemble_out(results, masked):
    out = np.empty((B, S, D), dtype=np.float32)
    for c in range(NCORES):
        b, l = divmod(c, 4)
        gA, gB = l, 7 - l
        res = results[c]["out"]
        out[b, 128 * gA:128 * gA + 128] = res[:, 0:128]
        out[b, 128 * gB:128 * gB + 128] = res[:, 128:256]
    return out


def kernel(x, Wq, bq, Wk, bk, Wv, bv, apply_causal_mask):
    x = np.ascontiguousarray(np.asarray(x, dtype=np.float32))
    Wq = np.asarray(Wq, dtype=np.float32)
    Wk = np.asarray(Wk, dtype=np.float32)
    Wv = np.asarray(Wv, dtype=np.float32)
    bq = np.asarray(bq, dtype=np.float32)
    bk = np.asarray(bk, dtype=np.float32)
    bv = np.asarray(bv, dtype=np.float32)
    masked = bool(int(np.asarray(apply_causal_mask)))

    nc = _get_program(masked)
    in_maps = build_in_maps(x, Wq, bq, Wk, bk, Wv, bv, masked)
    res = run_bass_kernel_spmd(nc, in_maps, list(range(NCORES))).results
    return assemble_out(res, masked)

